# revision 24
# baseline (speedup 1.0000x reference)
"""Trainium2 Bass kernel for nn_AudioImaginationForGLUE.

Pure data-parallel across 8 NeuronCores: each core handles 4 samples
(B=32 / 8). Inside a core, the two spans are processed as two sequential
phases (span 1 may read hidden-state rows written by span 0).

Math transformations (validated vs reference):
  - audio-MLP second layer folded into K/V projections:
       wk_eff = mlp_w2 @ wk,  wv_eff = mlp_w2 @ wv
  - key bias dropped (softmax shift invariance along key axis)
  - value bias folded into output-proj bias (softmax rows sum to 1):
       bo_eff = (mlp_b2 @ wv + bv) @ wo + bo
  - attention scale + all fp8 dequant scales folded into wq, bq
  - softmax max-subtraction dropped (scores are O(0.1): exp is safe)
  - softmax normalization applied to ctx rows instead of att matrix
  - ragged span handled by indirect-DMA gather/scatter with host-computed
    row indices; write-back is  gathered + wmask * (fused - gathered)
    so invalid rows are rewritten unchanged.

Precision plan (rel-err budget 2e-2; emulated absmax ~0.066 vs 0.108):
  nearly all matmuls fp8-e4m3 with DoubleRow perf mode (2 K-tiles per
  matmul at 0.5 cyc/row). Weights pre-scaled on host; activations
  re-scaled into fp8 range at eviction; all dequants folded into the
  next eviction's scale or into host-folded biases. Scores q@k stay
  bf16 (block-diagonal trick is incompatible with DoubleRow's K-sum);
  residual adds, LN stats, and the final write-back stay bf16/fp32.

Layout/schedule: activations transposed [feature->partitions,
tokens->free]. All weights are pre-rearranged on the HOST into the
[partition, chunk, free] SBUF layout so every weight DMA is contiguous
(strided rearrange loads cost ~10x in DMA descriptors). Span-0 gathers
read hs_in (no wait on the hs_in->hs_out bulk copy, which is emitted
late, right before the scatters that need it). Span-1 gather + span
build + q-projection interleave into span-0's stage-B tail behind the
per-pair scatters. Audio arrives pre-transposed via XBAR DMA-transpose
of host-packed fp8 token-pairs (uint16), split across both DMA queues.
FFN weights are SBUF-resident (fp8).
"""

import os
import numpy as np
import ml_dtypes

import concourse.bass as bass
import concourse.mybir as mybir
import concourse.tile as tile
from concourse import bacc
from concourse.masks import make_identity
from concourse.bass_utils import run_bass_kernel_spmd

F32 = mybir.dt.float32
BF16 = mybir.dt.bfloat16
FP8 = mybir.dt.float8e4
U16 = mybir.dt.uint16
I32 = mybir.dt.int32
AF = mybir.ActivationFunctionType
AX = mybir.AxisListType
OP = mybir.AluOpType
DR = mybir.MatmulPerfMode.DoubleRow
# CoreSim lacks Gelu; sim_test swaps in Tanh (mirrored in its reference)
GELU = AF.Tanh if os.environ.get("BASS_SIM_GELU_SWAP") else AF.Gelu

P = 128
B, S, H, NH, FF, A, TA, NSPAN, MAXL = 32, 512, 768, 12, 3072, 768, 1024, 2, 64
DH = H // NH          # 64
HC = H // P           # 6 hidden chunks
KP = HC // 2          # 3 double-K tile pairs
FC = FF // P          # 24 ffn chunks
TT = TA // P          # 8 audio token tiles
NCORES = 8
BPC = B // NCORES     # 4 samples per core
NPAIR = BPC // 2      # sample pairs (gather/scatter at 128 rows)
TBLK = 512            # audio token block for XBAR staging
NBLK = TA // TBLK
NB = BPC * MAXL       # 256, stage-B token width
SCALE = 1.0 / float(np.sqrt(DH))

# fp8 scale folding
S_W1 = 16.0             # mlp_w1 pre-scale
S_H1 = 32.0             # h1 storage scale
S_KV = 32.0             # wk_eff / wv_eff pre-scale
DEQ_KQ = 1.0 / (S_KV * S_H1)   # folded into wq/bq
S_WMM = 16.0            # generic fp8 weight pre-scale (wo/gaw/gtw/fw1/fw2)
S_SP = 4.0              # spanT/o1/o2 fp8 storage scale
S_WQ8 = float(2 ** 18)  # wq fp8 pre-scale (on top of SCALE*DEQ_KQ)
S_CTX = 32.0            # ctx fp8 storage scale
V8_SC = 0.25            # v stored fp8 at (S_KV*S_H1)*V8_SC = 256x true
DEQ_CTX = S_CTX / (S_KV * S_H1 * V8_SC)   # pc -> ctx8 eviction scale


def build_program():
    nc = bacc.Bacc("TRN2", target_bir_lowering=False, debug=False)

    t = {}
    t["hs_in"] = nc.dram_tensor("hs_in", [BPC * S, H], F32, kind="ExternalInput")
    t["audio"] = nc.dram_tensor("audio", [BPC, NSPAN, TA // 2, A], U16,
                                kind="ExternalInput")
    # host-rearranged resident weights: [P, chunk, free] contiguous
    for nm in ("w_mw1", "w_wk", "w_wv"):
        t[nm] = nc.dram_tensor(nm, [P, HC, H], FP8, kind="ExternalInput")
    t["w_fw1"] = nc.dram_tensor("w_fw1", [P, HC, FF], FP8, kind="ExternalInput")
    t["w_fw2"] = nc.dram_tensor("w_fw2", [P, FC, H], BF16, kind="ExternalInput")
    # host-rearranged streamed weights: [co, P, ci, P] contiguous per chunk
    for nm in ("w_wq", "w_wo", "w_gaw", "w_gtw"):
        t[nm] = nc.dram_tensor(nm, [HC, P, HC, P], FP8, kind="ExternalInput")
    for nm in ("p_mb1", "p_bq", "p_fb2", "p_gb", "p_g1", "p_b1", "p_g2", "p_b2"):
        t[nm] = nc.dram_tensor(nm, [P, HC], F32, kind="ExternalInput")
    t["p_fb1"] = nc.dram_tensor("p_fb1", [P, FC], F32, kind="ExternalInput")
    t["bo_row"] = nc.dram_tensor("bo_row", [1, H], BF16, kind="ExternalInput")
    t["ones_c"] = nc.dram_tensor("ones_c", [P, 1], BF16, kind="ExternalInput")
    t["ones_r"] = nc.dram_tensor("ones_r", [1, NB], BF16, kind="ExternalInput")
    # pair-packed gather indices and masks: [NSPAN, NPAIR, 128]
    t["gidx"] = nc.dram_tensor("gidx", [NSPAN, NPAIR, P], I32, kind="ExternalInput")
    t["vmsk"] = nc.dram_tensor("vmsk", [NSPAN, NPAIR, P], F32, kind="ExternalInput")
    t["wmsk"] = nc.dram_tensor("wmsk", [NSPAN, NPAIR, P], F32, kind="ExternalInput")
    t["hs_out"] = nc.dram_tensor("hs_out", [BPC * S, H], F32, kind="ExternalOutput")

    with tile.TileContext(nc) as tc, \
            nc.allow_low_precision("fp8/bf16 within 2e-2 rel-err budget"):
        _emit(nc, tc, t)
    nc.finalize()
    return nc


def _stage_audio(nc, t, perbs, s, b):
    """XBAR DMA-transpose one sample-span of packed-fp8 audio into SBUF.

    Blocks alternate between the two HWDGE queues (sync / scalar)."""
    aiT = perbs.tile([P, HC, TA // 2], U16, tag="aiT", bufs=2)
    for blk in range(NBLK):
        t2 = TBLK // 2
        eng = nc.sync if blk % 2 == 0 else nc.scalar
        for c in range(HC):
            eng.dma_start_transpose(
                out=aiT[:, c, blk * t2:(blk + 1) * t2],
                in_=t["audio"][b, s, blk * t2:(blk + 1) * t2,
                               c * P:(c + 1) * P])
    return aiT


def _gather_pair(nc, t, perbs, qa, ident, s, i, src, spanT, spanT8,
                 ptag="tp"):
    """Gather sample pair (2i, 2i+1) of span s from `src`, mask, and
    transpose into spanT (bf16) + spanT8 (fp8 x S_SP)."""
    gi2 = perbs.tile([P, 1], I32, tag="gi", bufs=4)
    nc.sync.dma_start(out=gi2[:],
                      in_=t["gidx"][s, i, :].rearrange("(p o) -> p o", o=1))
    vm2 = perbs.tile([P, 1], F32, tag="vm", bufs=4)
    nc.sync.dma_start(out=vm2[:],
                      in_=t["vmsk"][s, i, :].rearrange("(p o) -> p o", o=1))
    wm2 = perbs.tile([P, 1], F32, tag="wm", bufs=4)
    nc.sync.dma_start(out=wm2[:],
                      in_=t["wmsk"][s, i, :].rearrange("(p o) -> p o", o=1))
    gnat2 = perbs.tile([P, H], F32, tag="gnat", bufs=4)
    nc.gpsimd.indirect_dma_start(
        out=gnat2[:], out_offset=None, in_=src[:, :],
        in_offset=bass.IndirectOffsetOnAxis(ap=gi2[:, :1], axis=0))
    snat2 = perbs.tile([P, H], BF16, tag="snat", bufs=2)
    nc.vector.tensor_scalar_mul(snat2[:], gnat2[:], vm2[:, :1])
    pt = qa.tile([P, HC, P], BF16, tag=ptag, bufs=2)
    for c in range(HC):
        nc.tensor.transpose(out=pt[:, c, :],
                            in_=snat2[:, c * P:(c + 1) * P],
                            identity=ident[:, :])
    pr = pt[:, :, :].rearrange("p c (b l) -> p c b l", b=2)
    nc.scalar.copy(spanT[:, :, 2 * i:2 * i + 2, :], pr)
    nc.vector.tensor_scalar(out=spanT8[:, :, 2 * i:2 * i + 2, :], in0=pr,
                            scalar1=S_SP, scalar2=None, op0=OP.mult)
    return gnat2, wm2, gi2


def _qproj(nc, t, pa, qa, packs, spanT8, qT, mtag="mm", mbufs=2):
    """Batched q projection (fp8 DoubleRow) into block-diagonal layout."""
    nc.gpsimd.memset(qT[0:DH, :, :, 1, :], 0.0)
    nc.gpsimd.memset(qT[DH:P, :, :, 0, :], 0.0)
    for co in range(HC):
        wqc = pa.tile([P, HC, P], FP8, tag="wqc", bufs=2)
        nc.sync.dma_start(out=wqc[:], in_=t["w_wq"][co])
        pq = qa.tile([P, NB], F32, tag=mtag, bufs=mbufs)
        for kp in range(KP):
            nc.tensor.matmul(pq[:, :], wqc[:, 2 * kp:2 * kp + 2, :],
                             spanT8[:, 2 * kp:2 * kp + 2, :, :],
                             start=(kp == 0), stop=(kp == KP - 1),
                             perf_mode=DR)
        sc = 1.0 / (S_SP * S_WQ8)
        nc.scalar.activation(qT[0:DH, co, :, 0, :], pq[0:DH, :], AF.Identity,
                             scale=sc, bias=packs["p_bq"][0:DH, co:co + 1])
        nc.scalar.activation(qT[DH:P, co, :, 1, :], pq[DH:P, :], AF.Identity,
                             scale=sc, bias=packs["p_bq"][DH:P, co:co + 1])


def _emit(nc, tc, t):
    hs_in, hs_out = t["hs_in"], t["hs_out"]

    with (
        tc.tile_pool(name="const", bufs=1) as cpool,
        tc.tile_pool(name="resw", bufs=1) as resw,
        tc.tile_pool(name="perbs", bufs=1) as perbs,
    ):
        # ---- constants (scalar queue so sync stays clear early) ----
        ident = cpool.tile([P, P], BF16, tag="ident")
        make_identity(nc, ident)
        ones_col = cpool.tile([P, 1], BF16, tag="ones_col")
        nc.scalar.dma_start(out=ones_col[:], in_=t["ones_c"][:, :])
        ones_row = cpool.tile([1, NB], BF16, tag="ones_row")
        nc.scalar.dma_start(out=ones_row[:], in_=t["ones_r"][:, :])
        eps_t = cpool.tile([P, 1], F32, tag="eps_t")
        nc.vector.memset(eps_t[:], 1e-5)
        ones8 = cpool.tile([P, 1], FP8, tag="ones8")
        nc.vector.memset(ones8[:], 1.0)

        packs = {}
        for nm in ("p_mb1", "p_bq", "p_fb1", "p_fb2", "p_gb",
                   "p_g1", "p_b1", "p_g2", "p_b2"):
            nch = FC if nm == "p_fb1" else HC
            pk = cpool.tile([P, nch], F32, tag=nm)
            nc.scalar.dma_start(out=pk[:], in_=t[nm][:, :])
            packs[nm] = pk
        borow = cpool.tile([1, H], BF16, tag="borow")
        nc.scalar.dma_start(out=borow[:], in_=t["bo_row"][:, :])

        gn_t = [None] * NPAIR
        wm_t = [None] * NPAIR
        gi_t = [None] * NPAIR
        carry = {"ai": None, "spanT": None, "spanT8": None, "qT": None,
                 "gn": None}
        wres = {"ones8": ones8}

        for s in range(NSPAN):
            ctxT = perbs.tile([P, HC, BPC, MAXL], FP8, tag="ctxT")

            with (
                tc.tile_pool(name=f"sA{s}", bufs=1) as pa,
                tc.tile_pool(name=f"psA{s}", bufs=1, space="PSUM") as qa,
            ):
                if s == 0:
                    # startup-critical DMAs first: masks/gathers + audio b0
                    spanT = perbs.tile([P, HC, BPC, MAXL], BF16,
                                       tag="spanT", bufs=2)
                    spanT8 = perbs.tile([P, HC, BPC, MAXL], FP8,
                                        tag="spanT8", bufs=2)
                    ai_next = _stage_audio(nc, t, perbs, 0, 0)
                    for i in range(NPAIR):
                        gn_t[i], wm_t[i], gi_t[i] = _gather_pair(
                            nc, t, perbs, qa, ident, 0, i, hs_in,
                            spanT, spanT8)
                    # audio-side resident weights (contiguous loads)
                    for nm in ("mw1", "wk", "wv"):
                        ws = resw.tile([P, HC, H], FP8, tag="w_" + nm)
                        nc.sync.dma_start(out=ws[:], in_=t["w_" + nm][:])
                        wres[nm] = ws
                    qT = perbs.tile([P, HC, BPC, 2, MAXL], BF16,
                                    tag="qT", bufs=2)
                    _qproj(nc, t, pa, qa, packs, spanT8, qT)
                    # FFN resident weights on the scalar queue (needed
                    # only at stage B)
                    fw1 = resw.tile([P, HC, FF], FP8, tag="w_fw1")
                    nc.scalar.dma_start(out=fw1[:], in_=t["w_fw1"][:])
                    fw2 = resw.tile([P, FC, H], BF16, tag="w_fw2")
                    nc.scalar.dma_start(out=fw2[:], in_=t["w_fw2"][:])
                    wres["fw1"], wres["fw2"] = fw1, fw2
                else:
                    spanT, spanT8 = carry["spanT"], carry["spanT8"]
                    qT = carry["qT"]
                    gn_t = carry["gn"]
                    ai_next = carry["ai"]

                h1_cur = _prep_h1(nc, t, pa, qa, wres, packs, ai_next)
                v_cur = _prep_v(nc, pa, qa, wres, h1_cur)
                for b in range(BPC):
                    if b + 1 < BPC:
                        ai_next = _stage_audio(nc, t, perbs, s, b + 1)
                    nxt = {}

                    def cb(hp, b=b, ai=ai_next, nxt=nxt):
                        # fill softmax-latency gaps with next sample's prep
                        if b + 1 >= BPC:
                            return
                        if hp == 1:
                            nxt["h1"] = _prep_h1(nc, t, pa, qa, wres,
                                                 packs, ai)
                        elif hp == 3:
                            nxt["v"] = _prep_v(nc, pa, qa, wres, nxt["h1"])

                    _attn_sample(nc, t, s, b, pa, qa, wres, packs, ident,
                                 qT, ctxT, h1_cur, v_cur, cb)
                    if b + 1 < BPC:
                        h1_cur, v_cur = nxt["h1"], nxt["v"]

            with (
                tc.tile_pool(name=f"sB{s}", bufs=1) as pb,
                tc.tile_pool(name=f"psB{s}", bufs=1, space="PSUM") as qb,
            ):
                _stage_b(nc, t, s, pb, qb, perbs, wres, packs, ident,
                         ones_col, ones_row, eps_t, borow, spanT,
                         spanT8, ctxT, gn_t, wm_t, gi_t, hs_in, hs_out,
                         carry)


def _prep_h1(nc, t, pa, qa, wres, packs, aiT):
    """h1 = relu(ai @ mw1 + mb1), stored fp8 x32 (one sample)."""
    h1T = pa.tile([P, HC, TA], FP8, tag="h1T", bufs=2)
    for blk in range(NBLK):
        ai8 = aiT[:, :, blk * (TBLK // 2):(blk + 1) * (TBLK // 2)]
        for co in range(HC):
            ph = qa.tile([P, TBLK], F32, tag="mm", bufs=2)
            for kp in range(KP):
                nc.tensor.matmul(
                    ph[:, :],
                    wres["mw1"][:, 2 * kp:2 * kp + 2, co * P:(co + 1) * P],
                    ai8[:, 2 * kp:2 * kp + 2, :].bitcast(FP8),
                    start=(kp == 0), stop=(kp == KP - 1), perf_mode=DR)
            nc.scalar.activation(h1T[:, co, blk * TBLK:(blk + 1) * TBLK],
                                 ph[:, :], AF.Relu, scale=S_H1 / S_W1,
                                 bias=packs["p_mb1"][:, co:co + 1])
    return h1T


def _prep_v(nc, pa, qa, wres, h1T):
    """v = h1T.T @ wv_eff, stored fp8 at x256 true scale (one sample)."""
    v = pa.tile([P, TT, H], FP8, tag="v", bufs=2)
    for tt in range(TT):
        pv = qa.tile([P, 768], F32, tag="sc", bufs=2)
        for kp in range(KP):
            lhs = h1T[:, 2 * kp:2 * kp + 2, tt * P:(tt + 1) * P]
            nc.tensor.matmul(pv[:, 0:512], lhs,
                             wres["wv"][:, 2 * kp:2 * kp + 2, 0:512],
                             start=(kp == 0), stop=(kp == KP - 1), perf_mode=DR)
            nc.tensor.matmul(pv[:, 512:768], lhs,
                             wres["wv"][:, 2 * kp:2 * kp + 2, 512:768],
                             start=(kp == 0), stop=(kp == KP - 1), perf_mode=DR)
        nc.vector.tensor_scalar(out=v[:, tt, :], in0=pv[:, :], scalar1=V8_SC,
                                scalar2=None, op0=OP.mult)
    return v


def _attn_sample(nc, t, s, b, pa, qa, wres, packs, ident, qT, ctxT, h1T, v,
                 cb):
    """Attention for one sample (transposed-scores formulation). cb(hp)
    interleaves the next sample's prep matmuls into the dependency gaps
    left by the softmax chain."""

    def make_kc(hp):
        kc = pa.tile([P, TA], BF16, tag="kc", bufs=2)
        for nh in range(2):
            pk = qa.tile([P, 512], F32, tag="mm", bufs=2)
            for kp in range(KP):
                nc.tensor.matmul(
                    pk[:, :],
                    wres["wk"][:, 2 * kp:2 * kp + 2, hp * P:(hp + 1) * P],
                    h1T[:, 2 * kp:2 * kp + 2, nh * 512:(nh + 1) * 512],
                    start=(kp == 0), stop=(kp == KP - 1), perf_mode=DR)
            nc.vector.tensor_copy(kc[:, nh * 512:(nh + 1) * 512], pk[:, :])
        return kc

    ctx_nat = pa.tile([MAXL, H], BF16, tag="ctx_nat", bufs=1)
    ones8 = wres["ones8"]
    kc_cur = make_kc(0)
    for hp in range(NH // 2):
        # transposed scores: scoresT[token, query] = kc.T @ qT lands the
        # exp output directly in the [token, tt, query] layout the ctx
        # DoubleRow matmul wants - no PE transposes, no attT eviction
        psc = qa.tile([P, TT, P], F32, tag="sc", bufs=2)
        for tt in range(TT):
            nc.tensor.matmul(psc[:, tt, :],
                             kc_cur[:, tt * P:(tt + 1) * P],
                             qT[:, hp, b, :, :], start=True, stop=True)
        esbT = pa.tile([P, TT, P], FP8, tag="attT", bufs=1)
        # scores are O(0.1): exp never overflows, skip max-subtraction
        nc.scalar.activation(esbT[:, :, :], psc[:, :, :], AF.Exp)
        if hp + 1 < NH // 2:
            kc_cur = make_kc(hp + 1)
        cb(hp)
        # per-query sum over tokens (partition axis) via ones-matmul,
        # then a 1-row transpose turns 1/sum into a partition column
        ssum = qa.tile([1, P], F32, tag="tp", bufs=2)
        for tt in range(TT):
            nc.tensor.matmul(ssum[:, :], ones8[:, :], esbT[:, tt, :],
                             start=(tt == 0), stop=(tt == TT - 1))
        rrow = pa.tile([1, P], BF16, tag="rrow", bufs=2)
        nc.vector.reciprocal(rrow[:], ssum[:, :])
        prc = qa.tile([P, 1], BF16, tag="tp", bufs=2)
        nc.tensor.transpose(out=prc[:, :], in_=rrow[:1, :],
                            identity=ident[:1, :1])
        rec = pa.tile([P, 1], F32, tag="rec", bufs=2)
        nc.vector.tensor_copy(rec[:, :], prc[:, :])
        for hh in range(2):
            po = hh * DH
            h = 2 * hp + hh
            pc = qa.tile([MAXL, DH], F32, tag="tp", bufs=2)
            for tp in range(TT // 2):
                nc.tensor.matmul(
                    pc[:, :], esbT[:, 2 * tp:2 * tp + 2, po:po + DH],
                    v[:, 2 * tp:2 * tp + 2, h * DH:(h + 1) * DH],
                    start=(tp == 0), stop=(tp == TT // 2 - 1), perf_mode=DR)
            nc.vector.tensor_scalar(out=ctx_nat[:, h * DH:(h + 1) * DH],
                                    in0=pc[:, :], scalar1=rec[po:po + DH, :1],
                                    scalar2=DEQ_CTX, op0=OP.mult, op1=OP.mult)

    # ---- transpose ctx -> ctxT[:, :, b, :] (one combined eviction) ----
    pt = qa.tile([P, HC, MAXL], BF16, tag="tp", bufs=2)
    for c in range(HC):
        nc.tensor.transpose(out=pt[:, c, :],
                            in_=ctx_nat[:, c * P:(c + 1) * P],
                            identity=ident[:MAXL, :MAXL])
    nc.scalar.copy(ctxT[:, :, b, :], pt[:, :, :])


def _layernorm_T(nc, qb, pb, xT, outT, out8, gpack, bpack, ones_col,
                 ones_row, eps_t):
    """LayerNorm over the partition (feature) axis of xT [128, HC, BPC, MAXL].

    Column stats via ones-matmul; rstd computed on the [1, NB] row as
    exp(-0.5*ln(var+eps)) (Ln/Exp share the activation table with
    Exp/Relu/Square - no table reload); mean/rstd broadcast to 128
    partitions via rank-1 matmuls and read straight from PSUM by the
    norm ops. Writes bf16 outT and fp8 out8 = outT * S_SP (Act)."""
    psum = qb.tile([1, NB], F32, tag="st", bufs=2)
    for c in range(HC):
        nc.tensor.matmul(psum[:, :], ones_col[:, :], xT[:, c, :, :],
                         start=(c == 0), stop=(c == HC - 1))
    m_row = pb.tile([1, NB], BF16, tag="m_row", bufs=1)
    nc.vector.tensor_scalar_mul(m_row[:], psum[:, :], 1.0 / H)

    sq = pb.tile([P, HC, NB], BF16, tag="sq", bufs=1)
    for c in range(HC):
        nc.scalar.activation(sq[:, c, :], xT[:, c, :, :], AF.Square)
    psq = qb.tile([1, NB], F32, tag="st", bufs=2)
    for c in range(HC):
        nc.tensor.matmul(psq[:, :], ones_col[:, :], sq[:, c, :],
                         start=(c == 0), stop=(c == HC - 1))
    var = pb.tile([1, NB], F32, tag="var", bufs=1)
    msq = pb.tile([1, NB], F32, tag="msq", bufs=1)
    nc.scalar.activation(msq[:], m_row[:], AF.Square)
    nc.vector.tensor_scalar(out=var[:], in0=psq[:, :], scalar1=1.0 / H,
                            scalar2=None, op0=OP.mult)
    nc.vector.tensor_tensor(out=var[:], in0=var[:], in1=msq[:],
                            op=OP.subtract)
    # rstd row = exp(-0.5 * ln(var + eps))
    lnv = pb.tile([1, NB], F32, tag="lnv", bufs=1)
    nc.scalar.activation(lnv[:], var[:], AF.Ln, bias=eps_t[:1, :1])
    r_row = pb.tile([1, NB], BF16, tag="r_row", bufs=1)
    nc.scalar.activation(r_row[:], lnv[:], AF.Exp, scale=-0.5)
    # broadcast mean and rstd to all partitions via rank-1 matmuls
    pm_b = qb.tile([P, NB], F32, tag="st", bufs=2)
    nc.tensor.matmul(pm_b[:, :], ones_row[:1, :P], m_row[:1, :],
                     start=True, stop=True)
    pr_b = qb.tile([P, NB], F32, tag="st", bufs=2)
    nc.tensor.matmul(pr_b[:, :], ones_row[:1, :P], r_row[:1, :],
                     start=True, stop=True)
    for c in range(HC):
        nc.vector.tensor_tensor(out=outT[:, c, :, :], in0=xT[:, c, :, :],
                                in1=pm_b[:, :], op=OP.subtract)
        nc.vector.tensor_tensor(out=outT[:, c, :, :], in0=outT[:, c, :, :],
                                in1=pr_b[:, :], op=OP.mult)
        nc.vector.tensor_scalar(out=outT[:, c, :, :], in0=outT[:, c, :, :],
                                scalar1=gpack[:, c:c + 1],
                                scalar2=bpack[:, c:c + 1],
                                op0=OP.mult, op1=OP.add)
        nc.scalar.activation(out8[:, c, :, :], outT[:, c, :, :], AF.Identity,
                             scale=S_SP)


def _stage_b(nc, t, s, pb, qb, perbs, wres, packs, ident, ones_col,
             ones_row, eps_t, borow, spanT, spanT8, ctxT, gn_t, wm_t, gi_t,
             hs_in, hs_out, carry):
    """Batched (over b) fusion tail: o-proj, LN1, FFN, LN2, gates, merge.

    The tail interleaves span s+1's gathers/span-build/q-projection right
    behind each pair's scatter."""

    # ---- o = ctx @ wo + bo  (+ residual span) -> x1 ----
    x1 = pb.tile([P, HC, BPC, MAXL], BF16, tag="xT", bufs=2)
    for co in range(HC):
        wc = pb.tile([P, HC, P], FP8, tag="wcol", bufs=3)
        nc.sync.dma_start(out=wc[:], in_=t["w_wo"][co])
        po = qb.tile([P, NB], F32, tag="mmB", bufs=3)
        for kp in range(KP):
            nc.tensor.matmul(po[:, :], wc[:, 2 * kp:2 * kp + 2, :],
                             ctxT[:, 2 * kp:2 * kp + 2, :, :],
                             start=(kp == 0), stop=False, perf_mode=DR)
        nc.tensor.matmul(po[:, :], borow[:1, co * P:(co + 1) * P],
                         ones_row[:1, :], start=False, stop=True)
        nc.vector.tensor_scalar(out=x1[:, co, :, :], in0=po[:, :],
                                scalar1=1.0 / (S_CTX * S_WMM), scalar2=None,
                                op0=OP.mult)
        nc.vector.tensor_tensor(out=x1[:, co, :, :], in0=x1[:, co, :, :],
                                in1=spanT[:, co, :, :], op=OP.add)

    if s + 1 < NSPAN:
        carry["ai"] = _stage_audio(nc, t, perbs, s + 1, 0)
    if s == 0:
        # hs_in -> hs_out bulk copy: must precede the scatters below;
        # emitted here so it never competes with startup DMA
        rows = BPC * S
        step = rows // 8
        for i in range(8):
            eng = nc.sync if i % 2 == 0 else nc.scalar
            eng.dma_start(out=hs_out[i * step:(i + 1) * step, :],
                          in_=hs_in[i * step:(i + 1) * step, :])

    # ---- LN1 ----
    o1 = pb.tile([P, HC, BPC, MAXL], BF16, tag="out1T", bufs=1)
    o18 = pb.tile([P, HC, BPC, MAXL], FP8, tag="o18", bufs=1)
    _layernorm_T(nc, qb, pb, x1, o1, o18, packs["p_g1"], packs["p_b1"],
                 ones_col, ones_row, eps_t)

    # ---- FFN (fp8 DR): all 24 hf chunks resident, single-psum FFN2 ----
    hf = pb.tile([P, FC, NB], BF16, tag="hf", bufs=1)
    for cf in range(FC):
        ph = qb.tile([P, NB], F32, tag="mmB", bufs=3)
        for kp in range(KP):
            nc.tensor.matmul(ph[:, :],
                             wres["fw1"][:, 2 * kp:2 * kp + 2,
                                         cf * P:(cf + 1) * P],
                             o18[:, 2 * kp:2 * kp + 2, :, :],
                             start=(kp == 0), stop=(kp == KP - 1),
                             perf_mode=DR)
        nc.scalar.activation(hf[:, cf, :], ph[:, :], GELU,
                             scale=1.0 / (S_SP * S_WMM),
                             bias=packs["p_fb1"][:, cf:cf + 1])
    x2 = pb.tile([P, HC, BPC, MAXL], BF16, tag="xT", bufs=2)
    for co in range(HC):
        pacc = qb.tile([P, NB], F32, tag="mmB", bufs=3)
        for cf in range(FC):
            nc.tensor.matmul(pacc[:, :],
                             wres["fw2"][:, cf, co * P:(co + 1) * P],
                             hf[:, cf, :],
                             start=(cf == 0), stop=(cf == FC - 1))
        nc.vector.tensor_scalar(out=x2[:, co, :, :], in0=pacc[:, :],
                                scalar1=packs["p_fb2"][:, co:co + 1],
                                scalar2=None, op0=OP.add)
        nc.vector.tensor_tensor(out=x2[:, co, :, :], in0=x2[:, co, :, :],
                                in1=o1[:, co, :, :], op=OP.add)

    # ---- LN2 ----
    o2 = pb.tile([P, HC, BPC, MAXL], BF16, tag="out2T", bufs=1)
    o28 = pb.tile([P, HC, BPC, MAXL], FP8, tag="o28", bufs=1)
    _layernorm_T(nc, qb, pb, x2, o2, o28, packs["p_g2"], packs["p_b2"],
                 ones_col, ones_row, eps_t)

    # ---- gates (fp8 DR) ----
    gate = pb.tile([P, HC, BPC, MAXL], BF16, tag="gateT", bufs=1)
    for co in range(HC):
        wa = pb.tile([P, HC, P], FP8, tag="wcol", bufs=3)
        nc.sync.dma_start(out=wa[:], in_=t["w_gaw"][co])
        wt = pb.tile([P, HC, P], FP8, tag="wcol", bufs=3)
        nc.sync.dma_start(out=wt[:], in_=t["w_gtw"][co])
        pg = qb.tile([P, NB], F32, tag="mmB", bufs=3)
        for kp in range(KP):
            nc.tensor.matmul(pg[:, :], wa[:, 2 * kp:2 * kp + 2, :],
                             o28[:, 2 * kp:2 * kp + 2, :, :],
                             start=(kp == 0), stop=False, perf_mode=DR)
        for kp in range(KP):
            nc.tensor.matmul(pg[:, :], wt[:, 2 * kp:2 * kp + 2, :],
                             spanT8[:, 2 * kp:2 * kp + 2, :, :],
                             start=False, stop=(kp == KP - 1), perf_mode=DR)
        nc.scalar.activation(gate[:, co, :, :], pg[:, :], AF.Sigmoid,
                             scale=1.0 / (S_SP * S_WMM),
                             bias=packs["p_gb"][:, co:co + 1])

    # ---- fused = span + gate*(o2 - span), split per pair so each
    # pair's tail starts as soon as its half is done ----
    fused = []
    for i in range(NPAIR):
        fused_i = pb.tile([P, HC, 2, MAXL], BF16, tag=f"fused{i}", bufs=1)
        fused.append(fused_i)
    for i in range(NPAIR):
        sl = slice(2 * i, 2 * i + 2)
        nc.vector.tensor_tensor(out=fused[i][:, :, :, :],
                                in0=o2[:, :, sl, :],
                                in1=spanT[:, :, sl, :], op=OP.subtract)
        nc.vector.tensor_tensor(out=fused[i][:, :, :, :],
                                in0=fused[i][:, :, :, :],
                                in1=gate[:, :, sl, :], op=OP.mult)
        nc.vector.tensor_tensor(out=fused[i][:, :, :, :],
                                in0=fused[i][:, :, :, :],
                                in1=spanT[:, :, sl, :], op=OP.add)

    # ---- per-pair: back to natural, merge, scatter; then immediately
    # gather + build the same pair for span s+1 ----
    nspanT = nspanT8 = nqT = None
    ngn = [None] * NPAIR
    if s + 1 < NSPAN:
        nspanT = perbs.tile([P, HC, BPC, MAXL], BF16, tag="spanT", bufs=2)
        nspanT8 = perbs.tile([P, HC, BPC, MAXL], FP8, tag="spanT8", bufs=2)
        nqT = perbs.tile([P, HC, BPC, 2, MAXL], BF16, tag="qT", bufs=2)
    for i in range(NPAIR):
        fnat2 = pb.tile([P, H], BF16, tag="fnat", bufs=2)
        pt = qb.tile([P, HC, P], BF16, tag="tf", bufs=2)
        for c in range(HC):
            nc.tensor.transpose(
                out=pt[:, c, :],
                in_=fused[i][:, c, :, :].rearrange("p b l -> p (b l)"),
                identity=ident[:, :])
        nc.scalar.copy(fnat2[:, :], pt[:, :, :])
        merged2 = pb.tile([P, H], F32, tag="merged", bufs=2)
        nc.vector.tensor_tensor(out=merged2[:], in0=fnat2[:], in1=gn_t[i][:],
                                op=OP.subtract)
        nc.vector.tensor_scalar_mul(merged2[:], merged2[:], wm_t[i][:, :1])
        nc.vector.tensor_tensor(out=merged2[:], in0=merged2[:], in1=gn_t[i][:],
                                op=OP.add)
        nc.gpsimd.indirect_dma_start(
            out=hs_out[:, :],
            out_offset=bass.IndirectOffsetOnAxis(ap=gi_t[i][:, :1], axis=0),
            in_=merged2[:], in_offset=None)
        if s + 1 < NSPAN:
            ngn[i], wm_t[i], gi_t[i] = _gather_pair(
                nc, t, perbs, qb, ident, s + 1, i, hs_out, nspanT, nspanT8,
                ptag="tf")
    if s + 1 < NSPAN:
        _qproj(nc, t, pb, qb, packs, nspanT8, nqT, mtag="mmB", mbufs=3)
        carry["spanT"], carry["spanT8"] = nspanT, nspanT8
        carry["qT"], carry["gn"] = nqT, ngn


# ============================ host glue ============================

_NC_CACHE = None


def _get_program():
    global _NC_CACHE
    if _NC_CACHE is None:
        _NC_CACHE = build_program()
    return _NC_CACHE


def _res_layout(w, nch):
    """[K, N] -> [P, nch, N] resident SBUF layout, contiguous."""
    K, N = w.shape
    return np.ascontiguousarray(w.reshape(nch, P, N).transpose(1, 0, 2))


def _chunk_layout(w):
    """[H, H] -> [co, P, ci, P] streamed-chunk layout, contiguous."""
    return np.ascontiguousarray(
        w.reshape(HC, P, HC, P).transpose(2, 1, 0, 3))


def _fold_weights(inp):
    f64 = lambda x: np.asarray(x, np.float64)
    fp8 = ml_dtypes.float8_e4m3fn
    bf16 = ml_dtypes.bfloat16
    w = {}
    w["w_mw1"] = _res_layout((f64(inp["mlp_w1"]) * S_W1).astype(fp8), HC)
    wk_eff = f64(inp["mlp_w2"]) @ f64(inp["wk"])
    wv_eff = f64(inp["mlp_w2"]) @ f64(inp["wv"])
    w["w_wk"] = _res_layout((wk_eff * S_KV).astype(fp8), HC)
    w["w_wv"] = _res_layout((wv_eff * S_KV).astype(fp8), HC)
    bv_eff = f64(inp["mlp_b2"]) @ f64(inp["wv"]) + f64(inp["bv"])
    bo_eff = (bv_eff @ f64(inp["wo"]) + f64(inp["bo"]))
    w["w_wq"] = _chunk_layout(
        (f64(inp["wq"]) * SCALE * DEQ_KQ * S_WQ8).astype(fp8))
    bq_eff = (f64(inp["bq"]) * SCALE * DEQ_KQ).astype(np.float32)
    w["w_wo"] = _chunk_layout((f64(inp["wo"]) * S_WMM).astype(fp8))
    w["w_gaw"] = _chunk_layout((f64(inp["ga_w"]) * S_WMM).astype(fp8))
    w["w_gtw"] = _chunk_layout((f64(inp["gt_w"]) * S_WMM).astype(fp8))
    w["w_fw1"] = _res_layout((f64(inp["ffn_w1"]) * S_WMM).astype(fp8), HC)
    w["w_fw2"] = _res_layout(f64(inp["ffn_w2"]).astype(bf16), FC)
    gb_eff = (f64(inp["ga_b"]) + f64(inp["gt_b"])).astype(np.float32)

    def pack(vec, nch):
        return np.ascontiguousarray(
            np.asarray(vec, np.float32).reshape(nch, P).T)

    w["p_mb1"] = pack(np.asarray(inp["mlp_b1"], np.float64) * S_H1, HC)
    w["p_bq"] = pack(bq_eff, HC)
    w["p_fb1"] = pack(inp["ffn_b1"], FC)
    w["p_fb2"] = pack(inp["ffn_b2"], HC)
    w["p_gb"] = pack(gb_eff, HC)
    w["p_g1"] = pack(inp["ln1_g"], HC)
    w["p_b1"] = pack(inp["ln1_b"], HC)
    w["p_g2"] = pack(inp["ln2_g"], HC)
    w["p_b2"] = pack(inp["ln2_b"], HC)
    w["bo_row"] = (bo_eff * S_CTX * S_WMM).reshape(1, H).astype(bf16)
    w["ones_c"] = np.ones((P, 1), bf16)
    w["ones_r"] = np.ones((1, NB), bf16)
    return w


def _pack_audio(au):
    """fp8-cast audio then pack token pairs into uint16 little-endian so
    the 2-byte XBAR DMA transpose yields fp8 [feature, token] in SBUF."""
    a8 = np.asarray(au, np.float32).astype(ml_dtypes.float8_e4m3fn)
    a8 = np.ascontiguousarray(
        a8.reshape(BPC, NSPAN, TA // 2, 2, A).transpose(0, 1, 2, 4, 3))
    return a8.view(np.uint16).reshape(BPC, NSPAN, TA // 2, A)


def _span_meta(spans, active, core):
    ar = np.arange(MAXL)
    gidx = np.zeros((NSPAN, NPAIR, P), np.int32)
    vmsk = np.zeros((NSPAN, NPAIR, P), np.float32)
    wmsk = np.zeros((NSPAN, NPAIR, P), np.float32)
    for s in range(NSPAN):
        for bl in range(BPC):
            bg = core * BPC + bl
            st = int(spans[bg, s, 0])
            en = min(int(spans[bg, s, 1]), S)
            L = max(en - st, 0)
            idx = np.clip(st + ar, 0, S - 1)
            i, half = bl // 2, (bl % 2) * MAXL
            gidx[s, i, half:half + MAXL] = bl * S + idx
            vm = (ar < L).astype(np.float32)
            vmsk[s, i, half:half + MAXL] = vm
            wmsk[s, i, half:half + MAXL] = vm * np.float32(bool(active[bg, s]))
    return gidx, vmsk, wmsk


def _run(inputs, trace=False):
    nc = _get_program()
    hs = np.ascontiguousarray(inputs["hidden_states"], np.float32)
    au = np.asarray(inputs["audio_inputs"])
    spans = np.asarray(inputs["spans_token_pos"])
    active = np.asarray(inputs["in_audios"])
    w = _fold_weights(inputs)

    in_maps = []
    for c in range(NCORES):
        gidx, vmsk, wmsk = _span_meta(spans, active, c)
        m = dict(w)
        m["hs_in"] = hs[c * BPC:(c + 1) * BPC].reshape(BPC * S, H)
        m["audio"] = _pack_audio(au[c * BPC:(c + 1) * BPC])
        m["gidx"], m["vmsk"], m["wmsk"] = gidx, vmsk, wmsk
        in_maps.append(m)

    kw = {}
    if trace:
        kw = dict(trace=True, trace_cores=[0])
    res = run_bass_kernel_spmd(nc, in_maps, core_ids=list(range(NCORES)), **kw)
    out = np.empty((B, S, H), np.float32)
    for c in range(NCORES):
        out[c * BPC:(c + 1) * BPC] = res.results[c]["hs_out"].reshape(BPC, S, H)
    return out, res


def kernel(**inputs):
    out, _ = _run(inputs, trace=False)
    return out


# revision 26
# speedup vs baseline: 1.0745x; 1.0745x over previous
"""Trainium2 Bass kernel for nn_AudioImaginationForGLUE.

Pure data-parallel across 8 NeuronCores: each core handles 4 samples
(B=32 / 8). Inside a core, the two spans are processed as two sequential
phases (span 1 may read hidden-state rows written by span 0).

Math transformations (validated vs reference):
  - audio-MLP second layer folded into K/V projections:
       wk_eff = mlp_w2 @ wk,  wv_eff = mlp_w2 @ wv
  - key bias dropped (softmax shift invariance along key axis)
  - value bias folded into output-proj bias (softmax rows sum to 1):
       bo_eff = (mlp_b2 @ wv + bv) @ wo + bo
  - attention scale + all fp8 dequant scales folded into wq, bq
  - softmax max-subtraction dropped (scores are O(0.1): exp is safe)
  - softmax normalization applied to ctx rows instead of att matrix
  - ragged span handled by indirect-DMA gather/scatter with host-computed
    row indices; write-back is  gathered + wmask * (fused - gathered)
    so invalid rows are rewritten unchanged.

Precision plan (rel-err budget 2e-2; emulated absmax ~0.066 vs 0.108):
  nearly all matmuls fp8-e4m3 with DoubleRow perf mode (2 K-tiles per
  matmul at 0.5 cyc/row). Weights pre-scaled on host; activations
  re-scaled into fp8 range at eviction; all dequants folded into the
  next eviction's scale or into host-folded biases. Scores q@k stay
  bf16 (block-diagonal trick is incompatible with DoubleRow's K-sum);
  residual adds, LN stats, and the final write-back stay bf16/fp32.

Layout/schedule: activations transposed [feature->partitions,
tokens->free]. All weights are pre-rearranged on the HOST into the
[partition, chunk, free] SBUF layout so every weight DMA is contiguous
(strided rearrange loads cost ~10x in DMA descriptors). Span-0 gathers
read hs_in (no wait on the hs_in->hs_out bulk copy, which is emitted
late, right before the scatters that need it). Span-1 gather + span
build + q-projection interleave into span-0's stage-B tail behind the
per-pair scatters. Audio arrives pre-transposed via XBAR DMA-transpose
of host-packed fp8 token-pairs (uint16), split across both DMA queues.
FFN weights are SBUF-resident (fp8).
"""

import os
import numpy as np
import ml_dtypes

import concourse.bass as bass
import concourse.mybir as mybir
import concourse.tile as tile
from concourse import bacc
from concourse.masks import make_identity
from concourse.bass_utils import run_bass_kernel_spmd

F32 = mybir.dt.float32
BF16 = mybir.dt.bfloat16
FP8 = mybir.dt.float8e4
U16 = mybir.dt.uint16
I32 = mybir.dt.int32
AF = mybir.ActivationFunctionType
AX = mybir.AxisListType
OP = mybir.AluOpType
DR = mybir.MatmulPerfMode.DoubleRow
# CoreSim lacks Gelu; sim_test swaps in Tanh (mirrored in its reference)
GELU = AF.Tanh if os.environ.get("BASS_SIM_GELU_SWAP") else AF.Gelu

P = 128
B, S, H, NH, FF, A, TA, NSPAN, MAXL = 32, 512, 768, 12, 3072, 768, 1024, 2, 64
DH = H // NH          # 64
HC = H // P           # 6 hidden chunks
KP = HC // 2          # 3 double-K tile pairs
FC = FF // P          # 24 ffn chunks
TT = TA // P          # 8 audio token tiles
NCORES = 8
BPC = B // NCORES     # 4 samples per core
NPAIR = BPC // 2      # sample pairs (gather/scatter at 128 rows)
TBLK = 512            # audio token block for XBAR staging
NBLK = TA // TBLK
NB = BPC * MAXL       # 256, stage-B token width
SCALE = 1.0 / float(np.sqrt(DH))

# fp8 scale folding
S_W1 = 16.0             # mlp_w1 pre-scale
S_H1 = 32.0             # h1 storage scale
S_KV = 32.0             # wk_eff / wv_eff pre-scale
DEQ_KQ = 1.0 / (S_KV * S_H1)   # folded into wq/bq
S_WMM = 16.0            # generic fp8 weight pre-scale (wo/gaw/gtw/fw1/fw2)
S_SP = 4.0              # spanT/o1/o2 fp8 storage scale
S_WQ8 = float(2 ** 18)  # wq fp8 pre-scale (on top of SCALE*DEQ_KQ)
S_CTX = 32.0            # ctx fp8 storage scale
V8_SC = 0.25            # v stored fp8 at (S_KV*S_H1)*V8_SC = 256x true
DEQ_CTX = S_CTX / (S_KV * S_H1 * V8_SC)   # pc -> ctx8 eviction scale


def build_program():
    nc = bacc.Bacc("TRN2", target_bir_lowering=False, debug=False)

    t = {}
    t["hs_in"] = nc.dram_tensor("hs_in", [BPC * S, H], F32, kind="ExternalInput")
    t["audio"] = nc.dram_tensor("audio", [BPC, NSPAN, TA // 2, A], U16,
                                kind="ExternalInput")
    # host-rearranged resident weights: [P, chunk, free] contiguous
    for nm in ("w_mw1", "w_wk", "w_wv"):
        t[nm] = nc.dram_tensor(nm, [P, HC, H], FP8, kind="ExternalInput")
    t["w_fw1"] = nc.dram_tensor("w_fw1", [P, HC, FF], FP8, kind="ExternalInput")
    t["w_fw2"] = nc.dram_tensor("w_fw2", [P, FC, H], FP8, kind="ExternalInput")
    # host-rearranged streamed weights: [co, P, ci, P] contiguous per chunk
    for nm in ("w_wq", "w_wo", "w_gaw", "w_gtw"):
        t[nm] = nc.dram_tensor(nm, [HC, P, HC, P], FP8, kind="ExternalInput")
    for nm in ("p_mb1", "p_bq", "p_fb2", "p_gb", "p_g1", "p_b1", "p_g2", "p_b2"):
        t[nm] = nc.dram_tensor(nm, [P, HC], F32, kind="ExternalInput")
    t["p_fb1"] = nc.dram_tensor("p_fb1", [P, FC], F32, kind="ExternalInput")
    t["bo_row"] = nc.dram_tensor("bo_row", [1, H], BF16, kind="ExternalInput")
    t["ones_c"] = nc.dram_tensor("ones_c", [P, 1], BF16, kind="ExternalInput")
    t["ones_r"] = nc.dram_tensor("ones_r", [1, NB], BF16, kind="ExternalInput")
    # pair-packed gather indices and masks: [NSPAN, NPAIR, 128]
    t["gidx"] = nc.dram_tensor("gidx", [NSPAN, NPAIR, P], I32, kind="ExternalInput")
    t["vmsk"] = nc.dram_tensor("vmsk", [NSPAN, NPAIR, P], F32, kind="ExternalInput")
    t["wmsk"] = nc.dram_tensor("wmsk", [NSPAN, NPAIR, P], F32, kind="ExternalInput")
    t["hs_out"] = nc.dram_tensor("hs_out", [BPC * S, H], F32, kind="ExternalOutput")

    with tile.TileContext(nc) as tc, \
            nc.allow_low_precision("fp8/bf16 within 2e-2 rel-err budget"):
        _emit(nc, tc, t)
    nc.finalize()
    return nc


def _stage_audio(nc, t, perbs, s, b):
    """XBAR DMA-transpose one sample-span of packed-fp8 audio into SBUF.

    Blocks alternate between the two HWDGE queues (sync / scalar)."""
    aiT = perbs.tile([P, HC, TA // 2], U16, tag="aiT", bufs=2)
    for blk in range(NBLK):
        t2 = TBLK // 2
        eng = nc.sync if blk % 2 == 0 else nc.scalar
        for c in range(HC):
            eng.dma_start_transpose(
                out=aiT[:, c, blk * t2:(blk + 1) * t2],
                in_=t["audio"][b, s, blk * t2:(blk + 1) * t2,
                               c * P:(c + 1) * P])
    return aiT


def _gather_pair(nc, t, perbs, qa, ident, s, i, src, spanT, spanT8,
                 ptag="tp"):
    """Gather sample pair (2i, 2i+1) of span s from `src`, mask, and
    transpose into spanT (bf16) + spanT8 (fp8 x S_SP)."""
    gi2 = perbs.tile([P, 1], I32, tag="gi", bufs=4)
    nc.sync.dma_start(out=gi2[:],
                      in_=t["gidx"][s, i, :].rearrange("(p o) -> p o", o=1))
    vm2 = perbs.tile([P, 1], F32, tag="vm", bufs=4)
    nc.sync.dma_start(out=vm2[:],
                      in_=t["vmsk"][s, i, :].rearrange("(p o) -> p o", o=1))
    wm2 = perbs.tile([P, 1], F32, tag="wm", bufs=4)
    nc.sync.dma_start(out=wm2[:],
                      in_=t["wmsk"][s, i, :].rearrange("(p o) -> p o", o=1))
    gnat2 = perbs.tile([P, H], F32, tag="gnat", bufs=4)
    nc.gpsimd.indirect_dma_start(
        out=gnat2[:], out_offset=None, in_=src[:, :],
        in_offset=bass.IndirectOffsetOnAxis(ap=gi2[:, :1], axis=0))
    snat2 = perbs.tile([P, H], BF16, tag="snat", bufs=2)
    nc.vector.tensor_scalar_mul(snat2[:], gnat2[:], vm2[:, :1])
    pt = qa.tile([P, HC, P], BF16, tag=ptag, bufs=2)
    for c in range(HC):
        nc.tensor.transpose(out=pt[:, c, :],
                            in_=snat2[:, c * P:(c + 1) * P],
                            identity=ident[:, :])
    pr = pt[:, :, :].rearrange("p c (b l) -> p c b l", b=2)
    nc.scalar.copy(spanT[:, :, 2 * i:2 * i + 2, :], pr)
    nc.vector.tensor_scalar(out=spanT8[:, :, 2 * i:2 * i + 2, :], in0=pr,
                            scalar1=S_SP, scalar2=None, op0=OP.mult)
    return gnat2, wm2, gi2


def _qproj(nc, t, pa, qa, packs, spanT8, qT, mtag="mm", mbufs=2):
    """Batched q projection (fp8 DoubleRow) into block-diagonal layout."""
    nc.gpsimd.memset(qT[0:DH, :, :, 1, :], 0.0)
    nc.gpsimd.memset(qT[DH:P, :, :, 0, :], 0.0)
    for co in range(HC):
        wqc = pa.tile([P, HC, P], FP8, tag="wqc", bufs=2)
        nc.sync.dma_start(out=wqc[:], in_=t["w_wq"][co])
        pq = qa.tile([P, NB], F32, tag=mtag, bufs=mbufs)
        for kp in range(KP):
            nc.tensor.matmul(pq[:, :], wqc[:, 2 * kp:2 * kp + 2, :],
                             spanT8[:, 2 * kp:2 * kp + 2, :, :],
                             start=(kp == 0), stop=(kp == KP - 1),
                             perf_mode=DR)
        sc = 1.0 / (S_SP * S_WQ8)
        nc.scalar.activation(qT[0:DH, co, :, 0, :], pq[0:DH, :], AF.Identity,
                             scale=sc, bias=packs["p_bq"][0:DH, co:co + 1])
        nc.scalar.activation(qT[DH:P, co, :, 1, :], pq[DH:P, :], AF.Identity,
                             scale=sc, bias=packs["p_bq"][DH:P, co:co + 1])


def _emit(nc, tc, t):
    hs_in, hs_out = t["hs_in"], t["hs_out"]

    with (
        tc.tile_pool(name="const", bufs=1) as cpool,
        tc.tile_pool(name="resw", bufs=1) as resw,
        tc.tile_pool(name="perbs", bufs=1) as perbs,
    ):
        # ---- constants (scalar queue so sync stays clear early) ----
        ident = cpool.tile([P, P], BF16, tag="ident")
        make_identity(nc, ident)
        ones_col = cpool.tile([P, 1], BF16, tag="ones_col")
        nc.scalar.dma_start(out=ones_col[:], in_=t["ones_c"][:, :])
        ones_row = cpool.tile([1, NB], BF16, tag="ones_row")
        nc.scalar.dma_start(out=ones_row[:], in_=t["ones_r"][:, :])
        eps_t = cpool.tile([P, 1], F32, tag="eps_t")
        nc.vector.memset(eps_t[:], 1e-5)
        ones8 = cpool.tile([P, 1], FP8, tag="ones8")
        nc.vector.memset(ones8[:], 1.0)

        packs = {}
        for nm in ("p_mb1", "p_bq", "p_fb1", "p_fb2", "p_gb",
                   "p_g1", "p_b1", "p_g2", "p_b2"):
            nch = FC if nm == "p_fb1" else HC
            pk = cpool.tile([P, nch], F32, tag=nm)
            nc.scalar.dma_start(out=pk[:], in_=t[nm][:, :])
            packs[nm] = pk
        borow = cpool.tile([1, H], BF16, tag="borow")
        nc.scalar.dma_start(out=borow[:], in_=t["bo_row"][:, :])

        gn_t = [None] * NPAIR
        wm_t = [None] * NPAIR
        gi_t = [None] * NPAIR
        carry = {"ai": None, "spanT": None, "spanT8": None, "qT": None,
                 "gn": None}
        wres = {"ones8": ones8}

        for s in range(NSPAN):
            ctxT = perbs.tile([P, HC, BPC, MAXL], FP8, tag="ctxT")

            with (
                tc.tile_pool(name=f"sA{s}", bufs=1) as pa,
                tc.tile_pool(name=f"psA{s}", bufs=1, space="PSUM") as qa,
            ):
                if s == 0:
                    # startup-critical DMAs first: masks/gathers + audio b0
                    spanT = perbs.tile([P, HC, BPC, MAXL], BF16,
                                       tag="spanT", bufs=2)
                    spanT8 = perbs.tile([P, HC, BPC, MAXL], FP8,
                                        tag="spanT8", bufs=2)
                    ai_next = _stage_audio(nc, t, perbs, 0, 0)
                    for i in range(NPAIR):
                        gn_t[i], wm_t[i], gi_t[i] = _gather_pair(
                            nc, t, perbs, qa, ident, 0, i, hs_in,
                            spanT, spanT8)
                    # audio-side resident weights (contiguous loads)
                    for nm in ("mw1", "wk", "wv"):
                        ws = resw.tile([P, HC, H], FP8, tag="w_" + nm)
                        nc.sync.dma_start(out=ws[:], in_=t["w_" + nm][:])
                        wres[nm] = ws
                    qT = perbs.tile([P, HC, BPC, 2, MAXL], BF16,
                                    tag="qT", bufs=2)
                    _qproj(nc, t, pa, qa, packs, spanT8, qT)
                    # FFN resident weights on the scalar queue (needed
                    # only at stage B)
                    fw1 = resw.tile([P, HC, FF], FP8, tag="w_fw1")
                    nc.scalar.dma_start(out=fw1[:], in_=t["w_fw1"][:])
                    fw2 = resw.tile([P, FC, H], FP8, tag="w_fw2")
                    nc.scalar.dma_start(out=fw2[:], in_=t["w_fw2"][:])
                    wres["fw1"], wres["fw2"] = fw1, fw2
                else:
                    spanT, spanT8 = carry["spanT"], carry["spanT8"]
                    qT = carry["qT"]
                    gn_t = carry["gn"]
                    ai_next = carry["ai"]

                for b in range(BPC):
                    ai_cur = ai_next
                    if b + 1 < BPC:
                        ai_next = _stage_audio(nc, t, perbs, s, b + 1)
                    _stage_a(nc, t, s, b, pa, qa, wres, packs, ident,
                             qT, ctxT, ai_cur)

            with (
                tc.tile_pool(name=f"sB{s}", bufs=1) as pb,
                tc.tile_pool(name=f"psB{s}", bufs=1, space="PSUM") as qb,
            ):
                _stage_b(nc, t, s, pb, qb, perbs, wres, packs, ident,
                         ones_col, ones_row, eps_t, borow, spanT,
                         spanT8, ctxT, gn_t, wm_t, gi_t, hs_in, hs_out,
                         carry)


def _stage_a(nc, t, s, b, pa, qa, wres, packs, ident, qT, ctxT, aiT):
    """h1/V/K (fp8 DoubleRow) + attention for one sample."""

    # ---- h1 = relu(ai @ mw1 + mb1), stored fp8 x16 ----
    h1T = pa.tile([P, HC, TA], FP8, tag="h1T")
    for blk in range(NBLK):
        ai8 = aiT[:, :, blk * (TBLK // 2):(blk + 1) * (TBLK // 2)]
        for co in range(HC):
            ph = qa.tile([P, TBLK], F32, tag="mm", bufs=2)
            for kp in range(KP):
                nc.tensor.matmul(
                    ph[:, :],
                    wres["mw1"][:, 2 * kp:2 * kp + 2, co * P:(co + 1) * P],
                    ai8[:, 2 * kp:2 * kp + 2, :].bitcast(FP8),
                    start=(kp == 0), stop=(kp == KP - 1), perf_mode=DR)
            nc.scalar.activation(h1T[:, co, blk * TBLK:(blk + 1) * TBLK],
                                 ph[:, :], AF.Relu, scale=S_H1 / S_W1,
                                 bias=packs["p_mb1"][:, co:co + 1])

    # ---- v = h1T.T @ wv_eff, stored fp8 at x256 true scale ----
    v = pa.tile([P, TT, H], FP8, tag="v")
    for tt in range(TT):
        pv = qa.tile([P, 768], F32, tag="sc", bufs=2)
        for kp in range(KP):
            lhs = h1T[:, 2 * kp:2 * kp + 2, tt * P:(tt + 1) * P]
            nc.tensor.matmul(pv[:, 0:512], lhs,
                             wres["wv"][:, 2 * kp:2 * kp + 2, 0:512],
                             start=(kp == 0), stop=(kp == KP - 1), perf_mode=DR)
            nc.tensor.matmul(pv[:, 512:768], lhs,
                             wres["wv"][:, 2 * kp:2 * kp + 2, 512:768],
                             start=(kp == 0), stop=(kp == KP - 1), perf_mode=DR)
        nc.vector.tensor_scalar(out=v[:, tt, :], in0=pv[:, :], scalar1=V8_SC,
                                scalar2=None, op0=OP.mult)

    # ---- per head pair: kc produced one pair ahead so the PE fills the
    # softmax (ACT exp) latency with the next chunk's matmuls ----
    def make_kc(hp):
        kc = pa.tile([P, TA], BF16, tag="kc", bufs=2)
        for nh in range(2):
            pk = qa.tile([P, 512], F32, tag="mm", bufs=2)
            for kp in range(KP):
                nc.tensor.matmul(
                    pk[:, :],
                    wres["wk"][:, 2 * kp:2 * kp + 2, hp * P:(hp + 1) * P],
                    h1T[:, 2 * kp:2 * kp + 2, nh * 512:(nh + 1) * 512],
                    start=(kp == 0), stop=(kp == KP - 1), perf_mode=DR)
            nc.vector.tensor_copy(kc[:, nh * 512:(nh + 1) * 512], pk[:, :])
        return kc

    ctx_nat = pa.tile([MAXL, H], BF16, tag="ctx_nat", bufs=1)
    ones8 = wres["ones8"]
    kc_cur = make_kc(0)
    for hp in range(NH // 2):
        # transposed scores: scoresT[token, query] = kc.T @ qT lands the
        # exp output directly in the [token, tt, query] layout the ctx
        # DoubleRow matmul wants - no PE transposes, no attT eviction
        psc = qa.tile([P, TT, P], F32, tag="sc", bufs=2)
        for tt in range(TT):
            nc.tensor.matmul(psc[:, tt, :],
                             kc_cur[:, tt * P:(tt + 1) * P],
                             qT[:, hp, b, :, :], start=True, stop=True)
        esbT = pa.tile([P, TT, P], FP8, tag="attT", bufs=1)
        # scores are O(0.1): exp never overflows, skip max-subtraction
        nc.scalar.activation(esbT[:, :, :], psc[:, :, :], AF.Exp)
        if hp + 1 < NH // 2:
            kc_cur = make_kc(hp + 1)
        # per-query sum over tokens (partition axis) via ones-matmul,
        # then a 1-row transpose turns 1/sum into a partition column
        ssum = qa.tile([1, P], F32, tag="tp", bufs=2)
        for tt in range(TT):
            nc.tensor.matmul(ssum[:, :], ones8[:, :], esbT[:, tt, :],
                             start=(tt == 0), stop=(tt == TT - 1))
        rrow = pa.tile([1, P], BF16, tag="rrow", bufs=2)
        nc.vector.reciprocal(rrow[:], ssum[:, :])
        prc = qa.tile([P, 1], BF16, tag="tp", bufs=2)
        nc.tensor.transpose(out=prc[:, :], in_=rrow[:1, :],
                            identity=ident[:1, :1])
        rec = pa.tile([P, 1], F32, tag="rec", bufs=2)
        nc.vector.tensor_copy(rec[:, :], prc[:, :])
        for hh in range(2):
            po = hh * DH
            h = 2 * hp + hh
            pc = qa.tile([MAXL, DH], F32, tag="tp", bufs=2)
            for tp in range(TT // 2):
                nc.tensor.matmul(
                    pc[:, :], esbT[:, 2 * tp:2 * tp + 2, po:po + DH],
                    v[:, 2 * tp:2 * tp + 2, h * DH:(h + 1) * DH],
                    start=(tp == 0), stop=(tp == TT // 2 - 1), perf_mode=DR)
            nc.vector.tensor_scalar(out=ctx_nat[:, h * DH:(h + 1) * DH],
                                    in0=pc[:, :], scalar1=rec[po:po + DH, :1],
                                    scalar2=DEQ_CTX, op0=OP.mult, op1=OP.mult)

    # ---- transpose ctx -> ctxT[:, :, b, :] (one combined eviction) ----
    pt = qa.tile([P, HC, MAXL], BF16, tag="tp", bufs=2)
    for c in range(HC):
        nc.tensor.transpose(out=pt[:, c, :],
                            in_=ctx_nat[:, c * P:(c + 1) * P],
                            identity=ident[:MAXL, :MAXL])
    nc.scalar.copy(ctxT[:, :, b, :], pt[:, :, :])


def _layernorm_T(nc, qb, pb, xT, outT, out8, gpack, bpack, ones_col,
                 ones_row, eps_t):
    """LayerNorm over the partition (feature) axis of xT [128, HC, BPC, MAXL].

    Column stats via ones-matmul; rstd computed on the [1, NB] row as
    exp(-0.5*ln(var+eps)) (Ln/Exp share the activation table with
    Exp/Relu/Square - no table reload); mean/rstd broadcast to 128
    partitions via rank-1 matmuls and read straight from PSUM by the
    norm ops. Writes bf16 outT and fp8 out8 = outT * S_SP (Act)."""
    psum = qb.tile([1, NB], F32, tag="st", bufs=2)
    for c in range(HC):
        nc.tensor.matmul(psum[:, :], ones_col[:, :], xT[:, c, :, :],
                         start=(c == 0), stop=(c == HC - 1))
    m_row = pb.tile([1, NB], BF16, tag="m_row", bufs=1)
    nc.vector.tensor_scalar_mul(m_row[:], psum[:, :], 1.0 / H)

    sq = pb.tile([P, HC, NB], BF16, tag="sq", bufs=1)
    for c in range(HC):
        nc.scalar.activation(sq[:, c, :], xT[:, c, :, :], AF.Square)
    psq = qb.tile([1, NB], F32, tag="st", bufs=2)
    for c in range(HC):
        nc.tensor.matmul(psq[:, :], ones_col[:, :], sq[:, c, :],
                         start=(c == 0), stop=(c == HC - 1))
    var = pb.tile([1, NB], F32, tag="var", bufs=1)
    msq = pb.tile([1, NB], F32, tag="msq", bufs=1)
    nc.scalar.activation(msq[:], m_row[:], AF.Square)
    nc.vector.tensor_scalar(out=var[:], in0=psq[:, :], scalar1=1.0 / H,
                            scalar2=None, op0=OP.mult)
    nc.vector.tensor_tensor(out=var[:], in0=var[:], in1=msq[:],
                            op=OP.subtract)
    # rstd row = exp(-0.5 * ln(var + eps))
    lnv = pb.tile([1, NB], F32, tag="lnv", bufs=1)
    nc.scalar.activation(lnv[:], var[:], AF.Ln, bias=eps_t[:1, :1])
    r_row = pb.tile([1, NB], BF16, tag="r_row", bufs=1)
    nc.scalar.activation(r_row[:], lnv[:], AF.Exp, scale=-0.5)
    # broadcast mean and rstd to all partitions via rank-1 matmuls
    pm_b = qb.tile([P, NB], F32, tag="st", bufs=2)
    nc.tensor.matmul(pm_b[:, :], ones_row[:1, :P], m_row[:1, :],
                     start=True, stop=True)
    pr_b = qb.tile([P, NB], F32, tag="st", bufs=2)
    nc.tensor.matmul(pr_b[:, :], ones_row[:1, :P], r_row[:1, :],
                     start=True, stop=True)
    for c in range(HC):
        nc.vector.tensor_tensor(out=outT[:, c, :, :], in0=xT[:, c, :, :],
                                in1=pm_b[:, :], op=OP.subtract)
        nc.vector.tensor_tensor(out=outT[:, c, :, :], in0=outT[:, c, :, :],
                                in1=pr_b[:, :], op=OP.mult)
        nc.vector.tensor_scalar(out=outT[:, c, :, :], in0=outT[:, c, :, :],
                                scalar1=gpack[:, c:c + 1],
                                scalar2=bpack[:, c:c + 1],
                                op0=OP.mult, op1=OP.add)
        nc.scalar.activation(out8[:, c, :, :], outT[:, c, :, :], AF.Identity,
                             scale=S_SP)


def _stage_b(nc, t, s, pb, qb, perbs, wres, packs, ident, ones_col,
             ones_row, eps_t, borow, spanT, spanT8, ctxT, gn_t, wm_t, gi_t,
             hs_in, hs_out, carry):
    """Batched (over b) fusion tail: o-proj, LN1, FFN, LN2, gates, merge.

    The tail interleaves span s+1's gathers/span-build/q-projection right
    behind each pair's scatter."""

    # ---- o = ctx @ wo + bo  (+ residual span) -> x1 ----
    x1 = pb.tile([P, HC, BPC, MAXL], BF16, tag="xT", bufs=2)
    for co in range(HC):
        wc = pb.tile([P, HC, P], FP8, tag="wcol", bufs=3)
        nc.sync.dma_start(out=wc[:], in_=t["w_wo"][co])
        po = qb.tile([P, NB], F32, tag="mmB", bufs=3)
        for kp in range(KP):
            nc.tensor.matmul(po[:, :], wc[:, 2 * kp:2 * kp + 2, :],
                             ctxT[:, 2 * kp:2 * kp + 2, :, :],
                             start=(kp == 0), stop=False, perf_mode=DR)
        nc.tensor.matmul(po[:, :], borow[:1, co * P:(co + 1) * P],
                         ones_row[:1, :], start=False, stop=True)
        nc.vector.tensor_scalar(out=x1[:, co, :, :], in0=po[:, :],
                                scalar1=1.0 / (S_CTX * S_WMM), scalar2=None,
                                op0=OP.mult)
        nc.vector.tensor_tensor(out=x1[:, co, :, :], in0=x1[:, co, :, :],
                                in1=spanT[:, co, :, :], op=OP.add)

    if s + 1 < NSPAN:
        carry["ai"] = _stage_audio(nc, t, perbs, s + 1, 0)
    if s == 0:
        # hs_in -> hs_out bulk copy: must precede the scatters below;
        # emitted here so it never competes with startup DMA
        rows = BPC * S
        step = rows // 8
        for i in range(8):
            eng = nc.sync if i % 2 == 0 else nc.scalar
            eng.dma_start(out=hs_out[i * step:(i + 1) * step, :],
                          in_=hs_in[i * step:(i + 1) * step, :])

    # ---- LN1 ----
    o1 = pb.tile([P, HC, BPC, MAXL], BF16, tag="out1T", bufs=1)
    o18 = pb.tile([P, HC, BPC, MAXL], FP8, tag="o18", bufs=1)
    _layernorm_T(nc, qb, pb, x1, o1, o18, packs["p_g1"], packs["p_b1"],
                 ones_col, ones_row, eps_t)

    # ---- FFN (fp8 DR): all 24 hf chunks resident, single-psum FFN2 ----
    hf = pb.tile([P, FC, NB], FP8, tag="hf", bufs=1)
    for cf in range(FC):
        ph = qb.tile([P, NB], F32, tag="mmB", bufs=3)
        for kp in range(KP):
            nc.tensor.matmul(ph[:, :],
                             wres["fw1"][:, 2 * kp:2 * kp + 2,
                                         cf * P:(cf + 1) * P],
                             o18[:, 2 * kp:2 * kp + 2, :, :],
                             start=(kp == 0), stop=(kp == KP - 1),
                             perf_mode=DR)
        nc.scalar.activation(hf[:, cf, :], ph[:, :], GELU,
                             scale=1.0 / (S_SP * S_WMM),
                             bias=packs["p_fb1"][:, cf:cf + 1])
    x2 = pb.tile([P, HC, BPC, MAXL], BF16, tag="xT", bufs=2)
    for co in range(HC):
        pacc = qb.tile([P, NB], F32, tag="mmB", bufs=3)
        for fp in range(FC // 2):
            nc.tensor.matmul(pacc[:, :],
                             wres["fw2"][:, 2 * fp:2 * fp + 2,
                                         co * P:(co + 1) * P],
                             hf[:, 2 * fp:2 * fp + 2, :],
                             start=(fp == 0), stop=(fp == FC // 2 - 1),
                             perf_mode=DR)
        nc.vector.tensor_scalar(out=x2[:, co, :, :], in0=pacc[:, :],
                                scalar1=1.0 / S_WMM,
                                scalar2=packs["p_fb2"][:, co:co + 1],
                                op0=OP.mult, op1=OP.add)
        nc.vector.tensor_tensor(out=x2[:, co, :, :], in0=x2[:, co, :, :],
                                in1=o1[:, co, :, :], op=OP.add)

    # ---- LN2 ----
    o2 = pb.tile([P, HC, BPC, MAXL], BF16, tag="out2T", bufs=1)
    o28 = pb.tile([P, HC, BPC, MAXL], FP8, tag="o28", bufs=1)
    _layernorm_T(nc, qb, pb, x2, o2, o28, packs["p_g2"], packs["p_b2"],
                 ones_col, ones_row, eps_t)

    # ---- gates (fp8 DR) ----
    gate = pb.tile([P, HC, BPC, MAXL], BF16, tag="gateT", bufs=1)
    for co in range(HC):
        wa = pb.tile([P, HC, P], FP8, tag="wcol", bufs=3)
        nc.sync.dma_start(out=wa[:], in_=t["w_gaw"][co])
        wt = pb.tile([P, HC, P], FP8, tag="wcol", bufs=3)
        nc.sync.dma_start(out=wt[:], in_=t["w_gtw"][co])
        pg = qb.tile([P, NB], F32, tag="mmB", bufs=3)
        for kp in range(KP):
            nc.tensor.matmul(pg[:, :], wa[:, 2 * kp:2 * kp + 2, :],
                             o28[:, 2 * kp:2 * kp + 2, :, :],
                             start=(kp == 0), stop=False, perf_mode=DR)
        for kp in range(KP):
            nc.tensor.matmul(pg[:, :], wt[:, 2 * kp:2 * kp + 2, :],
                             spanT8[:, 2 * kp:2 * kp + 2, :, :],
                             start=False, stop=(kp == KP - 1), perf_mode=DR)
        nc.scalar.activation(gate[:, co, :, :], pg[:, :], AF.Sigmoid,
                             scale=1.0 / (S_SP * S_WMM),
                             bias=packs["p_gb"][:, co:co + 1])

    # ---- fused = span + gate*(o2 - span), split per pair so each
    # pair's tail starts as soon as its half is done ----
    fused = []
    for i in range(NPAIR):
        fused_i = pb.tile([P, HC, 2, MAXL], BF16, tag=f"fused{i}", bufs=1)
        fused.append(fused_i)
    for i in range(NPAIR):
        sl = slice(2 * i, 2 * i + 2)
        nc.vector.tensor_tensor(out=fused[i][:, :, :, :],
                                in0=o2[:, :, sl, :],
                                in1=spanT[:, :, sl, :], op=OP.subtract)
        nc.vector.tensor_tensor(out=fused[i][:, :, :, :],
                                in0=fused[i][:, :, :, :],
                                in1=gate[:, :, sl, :], op=OP.mult)
        nc.vector.tensor_tensor(out=fused[i][:, :, :, :],
                                in0=fused[i][:, :, :, :],
                                in1=spanT[:, :, sl, :], op=OP.add)

    # ---- per-pair: back to natural, merge, scatter; then immediately
    # gather + build the same pair for span s+1 ----
    nspanT = nspanT8 = nqT = None
    ngn = [None] * NPAIR
    if s + 1 < NSPAN:
        nspanT = perbs.tile([P, HC, BPC, MAXL], BF16, tag="spanT", bufs=2)
        nspanT8 = perbs.tile([P, HC, BPC, MAXL], FP8, tag="spanT8", bufs=2)
        nqT = perbs.tile([P, HC, BPC, 2, MAXL], BF16, tag="qT", bufs=2)
    for i in range(NPAIR):
        fnat2 = pb.tile([P, H], BF16, tag="fnat", bufs=2)
        pt = qb.tile([P, HC, P], BF16, tag="tf", bufs=2)
        for c in range(HC):
            nc.tensor.transpose(
                out=pt[:, c, :],
                in_=fused[i][:, c, :, :].rearrange("p b l -> p (b l)"),
                identity=ident[:, :])
        nc.scalar.copy(fnat2[:, :], pt[:, :, :])
        merged2 = pb.tile([P, H], F32, tag="merged", bufs=2)
        nc.vector.tensor_tensor(out=merged2[:], in0=fnat2[:], in1=gn_t[i][:],
                                op=OP.subtract)
        nc.vector.tensor_scalar_mul(merged2[:], merged2[:], wm_t[i][:, :1])
        nc.vector.tensor_tensor(out=merged2[:], in0=merged2[:], in1=gn_t[i][:],
                                op=OP.add)
        nc.gpsimd.indirect_dma_start(
            out=hs_out[:, :],
            out_offset=bass.IndirectOffsetOnAxis(ap=gi_t[i][:, :1], axis=0),
            in_=merged2[:], in_offset=None)
        if s + 1 < NSPAN:
            ngn[i], wm_t[i], gi_t[i] = _gather_pair(
                nc, t, perbs, qb, ident, s + 1, i, hs_out, nspanT, nspanT8,
                ptag="tf")
    if s + 1 < NSPAN:
        _qproj(nc, t, pb, qb, packs, nspanT8, nqT, mtag="mmB", mbufs=3)
        carry["spanT"], carry["spanT8"] = nspanT, nspanT8
        carry["qT"], carry["gn"] = nqT, ngn


# ============================ host glue ============================

_NC_CACHE = None


def _get_program():
    global _NC_CACHE
    if _NC_CACHE is None:
        _NC_CACHE = build_program()
    return _NC_CACHE


def _res_layout(w, nch):
    """[K, N] -> [P, nch, N] resident SBUF layout, contiguous."""
    K, N = w.shape
    return np.ascontiguousarray(w.reshape(nch, P, N).transpose(1, 0, 2))


def _chunk_layout(w):
    """[H, H] -> [co, P, ci, P] streamed-chunk layout, contiguous."""
    return np.ascontiguousarray(
        w.reshape(HC, P, HC, P).transpose(2, 1, 0, 3))


def _fold_weights(inp):
    f64 = lambda x: np.asarray(x, np.float64)
    fp8 = ml_dtypes.float8_e4m3fn
    bf16 = ml_dtypes.bfloat16
    w = {}
    w["w_mw1"] = _res_layout((f64(inp["mlp_w1"]) * S_W1).astype(fp8), HC)
    wk_eff = f64(inp["mlp_w2"]) @ f64(inp["wk"])
    wv_eff = f64(inp["mlp_w2"]) @ f64(inp["wv"])
    w["w_wk"] = _res_layout((wk_eff * S_KV).astype(fp8), HC)
    w["w_wv"] = _res_layout((wv_eff * S_KV).astype(fp8), HC)
    bv_eff = f64(inp["mlp_b2"]) @ f64(inp["wv"]) + f64(inp["bv"])
    bo_eff = (bv_eff @ f64(inp["wo"]) + f64(inp["bo"]))
    w["w_wq"] = _chunk_layout(
        (f64(inp["wq"]) * SCALE * DEQ_KQ * S_WQ8).astype(fp8))
    bq_eff = (f64(inp["bq"]) * SCALE * DEQ_KQ).astype(np.float32)
    w["w_wo"] = _chunk_layout((f64(inp["wo"]) * S_WMM).astype(fp8))
    w["w_gaw"] = _chunk_layout((f64(inp["ga_w"]) * S_WMM).astype(fp8))
    w["w_gtw"] = _chunk_layout((f64(inp["gt_w"]) * S_WMM).astype(fp8))
    w["w_fw1"] = _res_layout((f64(inp["ffn_w1"]) * S_WMM).astype(fp8), HC)
    w["w_fw2"] = _res_layout((f64(inp["ffn_w2"]) * S_WMM).astype(fp8), FC)
    gb_eff = (f64(inp["ga_b"]) + f64(inp["gt_b"])).astype(np.float32)

    def pack(vec, nch):
        return np.ascontiguousarray(
            np.asarray(vec, np.float32).reshape(nch, P).T)

    w["p_mb1"] = pack(np.asarray(inp["mlp_b1"], np.float64) * S_H1, HC)
    w["p_bq"] = pack(bq_eff, HC)
    w["p_fb1"] = pack(inp["ffn_b1"], FC)
    w["p_fb2"] = pack(inp["ffn_b2"], HC)
    w["p_gb"] = pack(gb_eff, HC)
    w["p_g1"] = pack(inp["ln1_g"], HC)
    w["p_b1"] = pack(inp["ln1_b"], HC)
    w["p_g2"] = pack(inp["ln2_g"], HC)
    w["p_b2"] = pack(inp["ln2_b"], HC)
    w["bo_row"] = (bo_eff * S_CTX * S_WMM).reshape(1, H).astype(bf16)
    w["ones_c"] = np.ones((P, 1), bf16)
    w["ones_r"] = np.ones((1, NB), bf16)
    return w


def _pack_audio(au):
    """fp8-cast audio then pack token pairs into uint16 little-endian so
    the 2-byte XBAR DMA transpose yields fp8 [feature, token] in SBUF."""
    a8 = np.asarray(au, np.float32).astype(ml_dtypes.float8_e4m3fn)
    a8 = np.ascontiguousarray(
        a8.reshape(BPC, NSPAN, TA // 2, 2, A).transpose(0, 1, 2, 4, 3))
    return a8.view(np.uint16).reshape(BPC, NSPAN, TA // 2, A)


def _span_meta(spans, active, core):
    ar = np.arange(MAXL)
    gidx = np.zeros((NSPAN, NPAIR, P), np.int32)
    vmsk = np.zeros((NSPAN, NPAIR, P), np.float32)
    wmsk = np.zeros((NSPAN, NPAIR, P), np.float32)
    for s in range(NSPAN):
        for bl in range(BPC):
            bg = core * BPC + bl
            st = int(spans[bg, s, 0])
            en = min(int(spans[bg, s, 1]), S)
            L = max(en - st, 0)
            idx = np.clip(st + ar, 0, S - 1)
            i, half = bl // 2, (bl % 2) * MAXL
            gidx[s, i, half:half + MAXL] = bl * S + idx
            vm = (ar < L).astype(np.float32)
            vmsk[s, i, half:half + MAXL] = vm
            wmsk[s, i, half:half + MAXL] = vm * np.float32(bool(active[bg, s]))
    return gidx, vmsk, wmsk


def _run(inputs, trace=False):
    nc = _get_program()
    hs = np.ascontiguousarray(inputs["hidden_states"], np.float32)
    au = np.asarray(inputs["audio_inputs"])
    spans = np.asarray(inputs["spans_token_pos"])
    active = np.asarray(inputs["in_audios"])
    w = _fold_weights(inputs)

    in_maps = []
    for c in range(NCORES):
        gidx, vmsk, wmsk = _span_meta(spans, active, c)
        m = dict(w)
        m["hs_in"] = hs[c * BPC:(c + 1) * BPC].reshape(BPC * S, H)
        m["audio"] = _pack_audio(au[c * BPC:(c + 1) * BPC])
        m["gidx"], m["vmsk"], m["wmsk"] = gidx, vmsk, wmsk
        in_maps.append(m)

    kw = {}
    if trace:
        kw = dict(trace=True, trace_cores=[0])
    res = run_bass_kernel_spmd(nc, in_maps, core_ids=list(range(NCORES)), **kw)
    out = np.empty((B, S, H), np.float32)
    for c in range(NCORES):
        out[c * BPC:(c + 1) * BPC] = res.results[c]["hs_out"].reshape(BPC, S, H)
    return out, res


def kernel(**inputs):
    out, _ = _run(inputs, trace=False)
    return out
